# revision 9
# baseline (speedup 1.0000x reference)
"""Biaffine span head kernel for Trainium2 (Bass/Tile), SPMD over 8 NeuronCores.

Math (per batch element b):
    Hu   = H @ U                        [L, d]
    a    = H @ w1 + b                   [L]
    c    = H @ w2                       [L]
    bil[l, off] = <Hu[l, :], H[l+off, :]>        off in [0, 30)
    s    = bil + a[l] + c[l+off]
    m    = mask[l] * mask_pad[l+off]
    out[l, off] = s if m != 0 else -1e9          [L, 30]

Sharding: data-parallel over batch B=8 across the 8 cores (one batch row per
core); the H@U GEMM and the band loop are fully local per shard, no
collectives.

Layout strategy on each core: everything is computed in a d-on-partitions
layout.  H^T is materialized via PE transposes; the big GEMM computes
HuT = (H @ U)^T directly (lhsT = U in natural layout, rhs = H^T), so its
output is already the stationary operand the band matmuls need.  The band is
computed as [128, 158] Gram blocks on the PE; the 30-wide diagonal band is
extracted through a small DRAM bounce using an affine access pattern with
row stride 159 (= 158 + 1), which turns the diagonal into a plain strided
DMA.
"""

import os
import sys

import numpy as np

for _p in ("/opt/trn_rl_repo",):
    if _p not in sys.path and os.path.isdir(_p):
        sys.path.insert(0, _p)

B = 8
L = 2048
D = 1024
K = 30          # band width (MAX_ANSWER_LEN)
P = 128         # partitions
NB = P + K      # 158: band matmul window
KC = D // P     # 8 contraction chunks
NBLK = 512      # GEMM-1 moving-dim block
LBN = L // NBLK  # 4
LT = L // P     # 16 l-tiles
LPAD = L + 32   # padded H^T width (2080)
NEG = -1.0e9

# fp32r runs the PE at 4x the fp32 rate for moving dims >= 256.
GEMM1_USE_F32R = True
# >1: wrap the compute phases in an on-device loop (perf measurement only)
PERF_ITERS = int(os.environ.get("BK_PERF_ITERS", "1"))

_CACHE = {}


def _build_nc():
    import concourse.bass as bass
    import concourse.tile as tile
    from concourse import bacc, mybir

    f32 = mybir.dt.float32
    i32 = mybir.dt.int32

    nc = bacc.Bacc("TRN2", target_bir_lowering=False, debug=False, num_devices=B)

    H_h = nc.dram_tensor("H", [L, D], f32, kind="ExternalInput")
    mask_h = nc.dram_tensor("mask", [L], i32, kind="ExternalInput")
    U_h = nc.dram_tensor("U", [D, D], f32, kind="ExternalInput")
    w_h = nc.dram_tensor("w", [2 * D], f32, kind="ExternalInput")
    b_h = nc.dram_tensor("b", [1], f32, kind="ExternalInput")
    out_h = nc.dram_tensor("out", [L, K], f32, kind="ExternalOutput")

    band_h = nc.dram_tensor("band_scratch", [LT, P, NB], f32)
    ac_h = nc.dram_tensor("ac_scratch", [2, LPAD], f32)      # row0 = a+b, row1 = c
    maskf_h = nc.dram_tensor("maskf_scratch", [LPAD], f32)

    ident_h = nc.inline_tensor(np.eye(P, dtype=np.float32), name="ident_const")

    H = H_h.ap()
    U = U_h.ap()
    out = out_h.ap()

    def dap(h, off, dims):
        # DRAM access pattern helper: dims = [(stride_elems, count), ...]
        return bass.AP(h, off, [list(d) for d in dims])

    with tile.TileContext(nc) as tc, tc.tile_pool(name="perm", bufs=1) as perm_pool:

        def perm(shape, dtype, name):
            return perm_pool.tile(shape, dtype, name=name, tag=name)

        # ---- persistent SBUF tensors ----
        ident_sb = perm([P, P], f32, name="ident_sb")
        nc.sync.dma_start(ident_sb[:], ident_h.ap())

        gdt = mybir.dt.float32r if GEMM1_USE_F32R else f32

        def rnd(ap):
            # view through which a producer rounds values to fp32r precision
            return ap.bitcast(gdt) if GEMM1_USE_F32R else ap

        U_sb = []
        with tc.tile_pool(name="ustage", bufs=2) as ustage_pool:
            for kk in range(KC):
                u_t = perm([P, D], f32, name=f"U_sb{kk}")
                if GEMM1_USE_F32R:
                    u_s = ustage_pool.tile([P, D], f32, name="u_s")
                    nc.sync.dma_start(u_s[:], U[kk * P:(kk + 1) * P, :])
                    nc.scalar.copy(rnd(u_t[:]), u_s[:])
                else:
                    nc.sync.dma_start(u_t[:], U[kk * P:(kk + 1) * P, :])
                U_sb.append(u_t)

        HT = []
        for kk in range(KC):
            ht_t = perm([P, LPAD], f32, name=f"HT{kk}")
            nc.gpsimd.memset(ht_t[:, L:LPAD], 0.0)
            HT.append(ht_t)

        HuT = []
        for kk in range(KC):
            hut_t = perm([P, L], f32, name=f"HuT{kk}")
            HuT.append(hut_t)

        # w repacked as per-chunk [128, 2] stationary blocks: [p, chunk, col]
        w12 = perm([P, KC, 2], f32, name="w12")
        nc.sync.dma_start(w12[:, :, 0], dap(w_h, 0, [(1, P), (P, KC)]))
        nc.sync.dma_start(w12[:, :, 1], dap(w_h, D, [(1, P), (P, KC)]))

        b_sb = perm([1, 1], f32, name="b_sb")
        nc.sync.dma_start(b_sb[:], dap(b_h, 0, [(1, 1), (1, 1)]))

        zpad = perm([2, 32], f32, name="zpad")
        nc.gpsimd.memset(zpad[:], 0.0)
        # zero pads of the padded DRAM vectors
        nc.sync.dma_start(dap(maskf_h, L, [(32, 1), (1, 32)]), zpad[0:1, :])
        nc.sync.dma_start(dap(ac_h, L, [(LPAD, 2), (1, 32)]), zpad[:, :])

        # ---- mask -> f32 in DRAM (cast once) ----
        with (
            tc.tile_pool(name="mcast", bufs=1) as mcast_pool,
        ):
            m_i = mcast_pool.tile([P, LT], i32, name="m_i")
            nc.sync.dma_start(m_i[:], dap(mask_h, 0, [(LT, P), (1, LT)]))
            m_f = mcast_pool.tile([P, LT], f32, name="m_f")
            nc.vector.tensor_copy(m_f[:], m_i[:])
            nc.sync.dma_start(dap(maskf_h, 0, [(LT, P), (1, LT)]), m_f[:])

        # ---- phases 1-4, optionally looped on-device for timing ----
        import contextlib

        loop_cm = (
            tc.For_i(0, PERF_ITERS, 1) if PERF_ITERS > 1 else contextlib.nullcontext()
        )
        with loop_cm:
            _phases(nc, tc, mybir, bass, dap, rnd, perm,
                    H, out, band_h, ac_h, maskf_h,
                    U_sb, HT, HuT, w12, b_sb, ident_sb)

    nc.compile()
    return nc


def _phases(nc, tc, mybir, bass, dap, rnd, perm,
            H, out, band_h, ac_h, maskf_h,
            U_sb, HT, HuT, w12, b_sb, ident_sb):
        f32 = mybir.dt.float32

        # ---- phase 1: H -> H^T via PE transposes ----
        with (
            tc.tile_pool(name="hstage", bufs=3) as hstage_pool,
            tc.tile_pool(name="trpsum", bufs=2, space="PSUM") as trps,
        ):
            for t in range(LT):
                l0 = t * P
                hstage = hstage_pool.tile([P, D], f32, name="hstage")
                nc.sync.dma_start(hstage[:], H[l0:l0 + P, :])
                for kk in range(KC):
                    tp = trps.tile([P, P], f32, name="tp")
                    nc.tensor.transpose(
                        tp[:], hstage[:, kk * P:(kk + 1) * P], ident_sb[:]
                    )
                    nc.scalar.copy(rnd(HT[kk][:, l0:l0 + P]), tp[:])

        # ---- phase 2: a/c thin GEMM: ac[2, L] = [w1 | w2].T @ H^T ----
        with (
            tc.tile_pool(name="acpsum", bufs=2, space="PSUM") as acps,
            tc.tile_pool(name="acsb", bufs=2) as acsb_pool,
        ):
            for lb in range(LBN):
                j0 = lb * NBLK
                acp = acps.tile([2, NBLK], f32, name="acp")
                for kk in range(KC):
                    nc.tensor.matmul(
                        acp[:],
                        lhsT=w12[:, kk, :],
                        rhs=HT[kk][:, j0:j0 + NBLK],
                        start=(kk == 0),
                        stop=(kk == KC - 1),
                    )
                ac_sb = acsb_pool.tile([2, NBLK], f32, name="ac_sb")
                nc.vector.tensor_copy(ac_sb[:], acp[:])
                # fold the scalar bias b into the a row
                nc.vector.tensor_scalar_add(ac_sb[0:1, :], ac_sb[0:1, :], b_sb[0:1, 0:1])
                nc.sync.dma_start(
                    dap(ac_h, j0, [(LPAD, 2), (1, NBLK)]), ac_sb[:]
                )

        # ---- phase 3: GEMM-1: HuT = U.T-free (lhsT=U) @ H^T ----
        with tc.tile_pool(name="hupsum", bufs=2, space="PSUM") as hups:
            for lb in range(LBN):
                j0 = lb * NBLK
                for dc in range(KC):
                    hp = hups.tile([P, NBLK], f32, name="hp")
                    for kk in range(KC):
                        nc.tensor.matmul(
                            hp[:],
                            lhsT=rnd(U_sb[kk][:, dc * P:(dc + 1) * P]),
                            rhs=rnd(HT[kk][:, j0:j0 + NBLK]),
                            start=(kk == 0),
                            stop=(kk == KC - 1),
                        )
                    nc.vector.tensor_copy(HuT[dc][:, j0:j0 + NBLK], hp[:])

        # ---- phase 4: band matmuls + diagonal extraction + fixup ----
        with (
            tc.tile_pool(name="bandpsum", bufs=2, space="PSUM") as bps,
            tc.tile_pool(name="bandsb", bufs=3) as bsb_pool,
            tc.tile_pool(name="fix", bufs=3) as fix_pool,
        ):
            for t in range(LT):
                l0 = t * P
                bp = bps.tile([P, NB], f32, name="bp")
                for kk in range(KC):
                    nc.tensor.matmul(
                        bp[:],
                        lhsT=HuT[kk][:, l0:l0 + P],
                        rhs=HT[kk][:, l0:l0 + NB],
                        start=(kk == 0),
                        stop=(kk == KC - 1),
                    )
                bsb = bsb_pool.tile([P, NB], f32, name="bsb")
                nc.scalar.copy(bsb[:], bp[:])
                nc.sync.dma_start(band_h.ap()[t], bsb[:])

                # diagonal band: bd[i, off] = G[i, i + off] -> stride NB+1
                bd = fix_pool.tile([P, K], f32, name="bd")
                nc.sync.dma_start(
                    bd[:], dap(band_h, t * P * NB, [(NB + 1, P), (1, K)])
                )
                a_col = fix_pool.tile([P, 1], f32, name="a_col")
                nc.sync.dma_start(a_col[:], dap(ac_h, l0, [(1, P), (1, 1)]))
                c_diag = fix_pool.tile([P, K], f32, name="c_diag")
                nc.sync.dma_start(c_diag[:], dap(ac_h, LPAD + l0, [(1, P), (1, K)]))
                mc = fix_pool.tile([P, 1], f32, name="mc")
                nc.sync.dma_start(mc[:], dap(maskf_h, l0, [(1, P), (1, 1)]))
                md = fix_pool.tile([P, K], f32, name="md")
                nc.sync.dma_start(md[:], dap(maskf_h, l0, [(1, P), (1, K)]))

                f1 = fix_pool.tile([P, K], f32, name="f1")
                # f1 = bil + a[l] + c[l+off]   (b already folded into a)
                nc.vector.scalar_tensor_tensor(
                    f1[:], in0=bd[:], scalar=a_col[:, 0:1], in1=c_diag[:],
                    op0=mybir.AluOpType.add, op1=mybir.AluOpType.add,
                )
                # f2 = f1 * mask[l] * mask[l+off]
                f2 = fix_pool.tile([P, K], f32, name="f2")
                nc.vector.scalar_tensor_tensor(
                    f2[:], in0=f1[:], scalar=mc[:, 0:1], in1=md[:],
                    op0=mybir.AluOpType.mult, op1=mybir.AluOpType.mult,
                )
                # f3 = m = mask[l] * mask[l+off]
                f3 = fix_pool.tile([P, K], f32, name="f3")
                nc.vector.tensor_scalar(
                    f3[:], in0=md[:], scalar1=mc[:, 0:1], scalar2=None,
                    op0=mybir.AluOpType.mult,
                )
                # f4 = (m - 1) * 1e9  ->  0 when kept, -1e9 when masked
                f4 = fix_pool.tile([P, K], f32, name="f4")
                nc.vector.tensor_scalar(
                    f4[:], in0=f3[:], scalar1=1.0, scalar2=-NEG,
                    op0=mybir.AluOpType.subtract, op1=mybir.AluOpType.mult,
                )
                o_t = fix_pool.tile([P, K], f32, name="o_t")
                nc.vector.tensor_add(o_t[:], f2[:], f4[:])
                nc.sync.dma_start(out[l0:l0 + P, :], o_t[:])


def get_nc():
    if "nc" not in _CACHE:
        _CACHE["nc"] = _build_nc()
    return _CACHE["nc"]


def kernel(H, attention_mask, U, w, b):
    from concourse.bass_utils import run_bass_kernel_spmd

    nc = get_nc()
    H = np.asarray(H, dtype=np.float32)
    attention_mask = np.asarray(attention_mask, dtype=np.int32)
    U_np = np.ascontiguousarray(np.asarray(U, dtype=np.float32))
    w_np = np.ascontiguousarray(np.asarray(w, dtype=np.float32).reshape(-1))
    b_np = np.ascontiguousarray(np.asarray(b, dtype=np.float32).reshape(-1))

    in_maps = []
    for i in range(B):
        in_maps.append({
            "H": np.ascontiguousarray(H[i]),
            "mask": np.ascontiguousarray(attention_mask[i]),
            "U": U_np,
            "w": w_np,
            "b": b_np,
        })
    res = run_bass_kernel_spmd(nc, in_maps, list(range(B)))
    return np.stack([res.results[i]["out"] for i in range(B)], axis=0)


# revision 14
# speedup vs baseline: 1.1767x; 1.1767x over previous
"""Biaffine span head kernel for Trainium2 (Bass/Tile), SPMD over 8 NeuronCores.

Math (per batch element b):
    Hu   = H @ U                        [L, d]
    a    = H @ w1 + b                   [L]
    c    = H @ w2                       [L]
    bil[l, off] = <Hu[l, :], H[l+off, :]>        off in [0, 30)
    s    = bil + a[l] + c[l+off]
    m    = mask[l] * mask_pad[l+off]
    out[l, off] = s if m != 0 else -1e9          [L, 30]

Sharding: data-parallel over batch B=8 across the 8 cores (one batch row per
core); the H@U GEMM and the band loop are fully local per shard, no
collectives.

Per-core schedule: a software pipeline over 4 l-blocks of 512. For block lb:
H tiles are DMA'd in, transposed on the PE (4 transposes grouped into one
PSUM bank, one batched [128,512] copy out), then HuT = (H@U)^T accumulates
with fp32r (TF32) matmuls at 1 cycle/row, then the band Gram blocks
[128,158] run in fp32, and the 30-wide diagonal is extracted via a DRAM
bounce whose read access pattern has row stride 159 (diagonal = affine in
DRAM address space). The mask/bias fixup runs one block behind so every
dependency points backward in program order.
"""

import os
import sys

import numpy as np

for _p in ("/opt/trn_rl_repo",):
    if _p not in sys.path and os.path.isdir(_p):
        sys.path.insert(0, _p)

B = 8
L = 2048
D = 1024
K = 30          # band width (MAX_ANSWER_LEN)
P = 128         # partitions
NB = P + K      # 158: band matmul window
KC = D // P     # 8 contraction chunks
NBLK = 512      # l-block width
LBN = L // NBLK  # 4
TPB = NBLK // P  # 4 l-tiles per block
LPAD = L + 32   # padded H^T width (2080)
NEG = -1.0e9

# fp32r (TF32) runs the PE at 4x the fp32 rate for moving dims >= 256.
GEMM1_USE_F32R = True
# >1: wrap the compute phases in an on-device loop (perf measurement only)
PERF_ITERS = int(os.environ.get("BK_PERF_ITERS", "1"))

_CACHE = {}


def _build_nc():
    import contextlib

    import concourse.bass as bass
    import concourse.tile as tile
    from concourse import bacc, mybir

    f32 = mybir.dt.float32
    i32 = mybir.dt.int32
    gdt = mybir.dt.float32r if GEMM1_USE_F32R else f32

    nc = bacc.Bacc("TRN2", target_bir_lowering=False, debug=False, num_devices=B)

    H_h = nc.dram_tensor("H", [L, D], f32, kind="ExternalInput")
    mask_h = nc.dram_tensor("mask", [L], i32, kind="ExternalInput")
    U_h = nc.dram_tensor("U", [D, D], f32, kind="ExternalInput")
    w_h = nc.dram_tensor("w", [2 * D], f32, kind="ExternalInput")
    b_h = nc.dram_tensor("b", [1], f32, kind="ExternalInput")
    out_h = nc.dram_tensor("out", [L, K], f32, kind="ExternalOutput")

    band_h = nc.dram_tensor("band_scratch", [L // P, P, NB], f32)
    ac_h = nc.dram_tensor("ac_scratch", [2, LPAD], f32)      # row0 = a+b, row1 = c
    maskf_h = nc.dram_tensor("maskf_scratch", [LPAD], f32)

    ident_h = nc.inline_tensor(np.eye(P, dtype=np.float32), name="ident_const")

    H = H_h.ap()
    U = U_h.ap()
    out = out_h.ap()

    def dap(h, off, dims):
        # DRAM access pattern helper: dims = [(stride_elems, count), ...]
        return bass.AP(h, off, [list(d) for d in dims])

    with tile.TileContext(nc) as tc, tc.tile_pool(name="perm", bufs=1) as perm_pool:

        def perm(shape, dtype, name):
            return perm_pool.tile(shape, dtype, name=name, tag=name)

        def rnd(ap):
            # view through which a producer rounds values to fp32r precision
            return ap.bitcast(gdt) if GEMM1_USE_F32R else ap

        # ---- persistent SBUF tensors ----
        ident_sb = perm([P, P], f32, name="ident_sb")
        nc.sync.dma_start(ident_sb[:], ident_h.ap())

        U_sb = [perm([P, D], f32, name=f"U_sb{kk}") for kk in range(KC)]
        HT = [perm([P, LPAD], f32, name=f"HT{kk}") for kk in range(KC)]
        HuT = [perm([P, L], f32, name=f"HuT{kk}") for kk in range(KC)]
        w12 = perm([P, KC, 2], f32, name="w12")
        b_sb = perm([1, 1], f32, name="b_sb")
        zpad = perm([2, 32], f32, name="zpad")

        for kk in range(KC):
            nc.gpsimd.memset(HT[kk][:, L:LPAD], 0.0)
        nc.gpsimd.memset(zpad[:], 0.0)
        nc.sync.dma_start(dap(maskf_h, L, [(32, 1), (1, 32)]), zpad[0:1, :])
        nc.sync.dma_start(dap(ac_h, L, [(LPAD, 2), (1, 32)]), zpad[:, :])
        nc.sync.dma_start(b_sb[:], dap(b_h, 0, [(1, 1), (1, 1)]))

        loop_cm = (
            tc.For_i(0, PERF_ITERS, 1) if PERF_ITERS > 1 else contextlib.nullcontext()
        )
        with loop_cm, contextlib.ExitStack() as ctx:
            hstage_pool = ctx.enter_context(tc.tile_pool(name="hstage", bufs=5))
            wstage_pool = ctx.enter_context(tc.tile_pool(name="wstage", bufs=2))
            trps = ctx.enter_context(tc.tile_pool(name="trpsum", bufs=2, space="PSUM"))
            hups = ctx.enter_context(tc.tile_pool(name="hupsum", bufs=3, space="PSUM"))
            bps = ctx.enter_context(tc.tile_pool(name="bandpsum", bufs=2, space="PSUM"))
            acps = ctx.enter_context(tc.tile_pool(name="acpsum", bufs=1, space="PSUM"))
            acsb_pool = ctx.enter_context(tc.tile_pool(name="acsb", bufs=2))
            bsb_pool = ctx.enter_context(tc.tile_pool(name="bandsb", bufs=3))
            fix_pool = ctx.enter_context(tc.tile_pool(name="fix", bufs=2))
            mcast_pool = ctx.enter_context(tc.tile_pool(name="mcast", bufs=1))

            hstages = {}

            def load_block(lb):
                for i in range(TPB):
                    l0 = (lb * TPB + i) * P
                    hs = hstage_pool.tile([P, D], f32, name="hs", tag="hs")
                    nc.sync.dma_start(hs[:], H[l0:l0 + P, :])
                    hstages[(lb, i)] = hs

            def transposes(lb):
                j0 = lb * NBLK
                for kk in range(KC):
                    tp = trps.tile([P, NBLK], f32, name="tp", tag="tp")
                    for i in range(TPB):
                        nc.tensor.matmul(
                            tp[:, i * P:(i + 1) * P],
                            lhsT=hstages[(lb, i)][:, kk * P:(kk + 1) * P],
                            rhs=ident_sb[:],
                            is_transpose=True,
                            start=(i == 0),
                            stop=(i == TPB - 1),
                        )
                    nc.scalar.copy(rnd(HT[kk][:, j0:j0 + NBLK]), tp[:])
                for i in range(TPB):
                    del hstages[(lb, i)]

            def setup_weights():
                # emitted after block-0 H loads so the H DMAs go out first
                for kk in range(KC):
                    u_s = wstage_pool.tile([P, D], f32, name="u_s", tag="u_s")
                    nc.scalar.dma_start(u_s[:], U[kk * P:(kk + 1) * P, :])
                    if GEMM1_USE_F32R:
                        nc.scalar.copy(rnd(U_sb[kk][:]), u_s[:])
                    else:
                        nc.vector.tensor_copy(U_sb[kk][:], u_s[:])
                w_s = wstage_pool.tile([P, KC, 2], f32, name="w_s", tag="w_s")
                nc.sync.dma_start(w_s[:, :, 0], dap(w_h, 0, [(1, P), (P, KC)]))
                nc.sync.dma_start(w_s[:, :, 1], dap(w_h, D, [(1, P), (P, KC)]))
                nc.scalar.copy(rnd(w12[:]), w_s[:])
                m_i = mcast_pool.tile([P, L // P], i32, name="m_i")
                nc.sync.dma_start(m_i[:], dap(mask_h, 0, [(L // P, P), (1, L // P)]))
                m_f = mcast_pool.tile([P, L // P], f32, name="m_f")
                nc.vector.tensor_copy(m_f[:], m_i[:])
                nc.sync.dma_start(
                    dap(maskf_h, 0, [(L // P, P), (1, L // P)]), m_f[:]
                )

            def gemm1(lb):
                j0 = lb * NBLK
                for dc in range(KC):
                    hp = hups.tile([P, NBLK], f32, name="hp", tag="hp")
                    for kk in range(KC):
                        nc.tensor.matmul(
                            hp[:],
                            lhsT=rnd(U_sb[kk][:, dc * P:(dc + 1) * P]),
                            rhs=rnd(HT[kk][:, j0:j0 + NBLK]),
                            start=(kk == 0),
                            stop=(kk == KC - 1),
                        )
                    nc.vector.tensor_copy(HuT[dc][:, j0:j0 + NBLK], hp[:])

            def ac_gemm(lb):
                j0 = lb * NBLK
                acp = acps.tile([2, NBLK], f32, name="acp", tag="acp")
                for kk in range(KC):
                    nc.tensor.matmul(
                        acp[:],
                        lhsT=rnd(w12[:, kk, :]),
                        rhs=rnd(HT[kk][:, j0:j0 + NBLK]),
                        start=(kk == 0),
                        stop=(kk == KC - 1),
                    )
                ac_sb = acsb_pool.tile([2, NBLK], f32, name="ac_sb", tag="ac_sb")
                nc.vector.tensor_copy(ac_sb[:], acp[:])
                nc.vector.tensor_scalar_add(
                    ac_sb[0:1, :], ac_sb[0:1, :], b_sb[0:1, 0:1]
                )
                nc.scalar.dma_start(dap(ac_h, j0, [(LPAD, 2), (1, NBLK)]), ac_sb[:])

            def band_block(lb):
                bsb = bsb_pool.tile([P, TPB, NB], f32, name="bsb", tag="bsb")
                for i in range(TPB):
                    l0 = (lb * TPB + i) * P
                    bp = bps.tile([P, NB], f32, name="bp", tag="bp")
                    for kk in range(KC):
                        nc.tensor.matmul(
                            bp[:],
                            lhsT=HuT[kk][:, l0:l0 + P],
                            rhs=HT[kk][:, l0:l0 + NB],
                            start=(kk == 0),
                            stop=(kk == KC - 1),
                        )
                    nc.vector.tensor_copy(bsb[:, i, :], bp[:])
                nc.scalar.dma_start(
                    dap(band_h, lb * TPB * P * NB, [(NB, P), (P * NB, TPB), (1, NB)]),
                    bsb[:],
                )

            def fixup_block(lb):
                j0 = lb * NBLK
                bd_blk = fix_pool.tile([P, TPB, K], f32, name="bd_blk", tag="bd_blk")
                nc.sync.dma_start(
                    bd_blk[:],
                    dap(band_h, lb * TPB * P * NB, [(NB + 1, P), (P * NB, TPB), (1, K)]),
                )
                a_blk = fix_pool.tile([P, TPB], f32, name="a_blk", tag="a_blk")
                nc.sync.dma_start(a_blk[:], dap(ac_h, j0, [(1, P), (P, TPB)]))
                c_blk = fix_pool.tile([P, TPB, K], f32, name="c_blk", tag="c_blk")
                nc.sync.dma_start(
                    c_blk[:], dap(ac_h, LPAD + j0, [(1, P), (P, TPB), (1, K)])
                )
                md_blk = fix_pool.tile([P, TPB, K], f32, name="md_blk", tag="md_blk")
                nc.sync.dma_start(
                    md_blk[:], dap(maskf_h, j0, [(1, P), (P, TPB), (1, K)])
                )
                o_blk = fix_pool.tile([P, TPB, K], f32, name="o_blk", tag="o_blk")
                for i in range(TPB):
                    bd = bd_blk[:, i, :]
                    a_col = a_blk[:, i:i + 1]
                    c_d = c_blk[:, i, :]
                    md = md_blk[:, i, :]
                    mc = md_blk[:, i, 0:1]
                    f1 = fix_pool.tile([P, K], f32, name="f1", tag="f1")
                    # f1 = bil + a[l] + c[l+off]   (b already folded into a)
                    nc.vector.scalar_tensor_tensor(
                        f1[:], in0=bd, scalar=a_col, in1=c_d,
                        op0=mybir.AluOpType.add, op1=mybir.AluOpType.add,
                    )
                    # f2 = f1 * mask[l] * mask[l+off]
                    f2 = fix_pool.tile([P, K], f32, name="f2", tag="f2")
                    nc.vector.scalar_tensor_tensor(
                        f2[:], in0=f1[:], scalar=mc, in1=md,
                        op0=mybir.AluOpType.mult, op1=mybir.AluOpType.mult,
                    )
                    # f3 = m;  f4 = (m - 1) * 1e9  (0 kept / -1e9 masked)
                    f3 = fix_pool.tile([P, K], f32, name="f3", tag="f3")
                    nc.vector.tensor_scalar(
                        f3[:], in0=md, scalar1=mc, scalar2=None,
                        op0=mybir.AluOpType.mult,
                    )
                    f4 = fix_pool.tile([P, K], f32, name="f4", tag="f4")
                    nc.vector.tensor_scalar(
                        f4[:], in0=f3[:], scalar1=1.0, scalar2=-NEG,
                        op0=mybir.AluOpType.subtract, op1=mybir.AluOpType.mult,
                    )
                    nc.vector.tensor_add(o_blk[:, i, :], f2[:], f4[:])
                nc.sync.dma_start(
                    dap(out_h, lb * NBLK * K, [(K, P), (P * K, TPB), (1, K)]),
                    o_blk[:],
                )

            # ---- pipeline ----
            load_block(0)
            setup_weights()
            transposes(0)
            for lb in range(LBN):
                if lb + 1 < LBN:
                    load_block(lb + 1)
                    transposes(lb + 1)
                gemm1(lb)
                ac_gemm(lb)
                band_block(lb)
                if lb > 0:
                    fixup_block(lb - 1)
            fixup_block(LBN - 1)

    nc.compile()
    return nc


def get_nc():
    if "nc" not in _CACHE:
        _CACHE["nc"] = _build_nc()
    return _CACHE["nc"]


def kernel(H, attention_mask, U, w, b):
    from concourse.bass_utils import run_bass_kernel_spmd

    nc = get_nc()
    H = np.asarray(H, dtype=np.float32)
    attention_mask = np.asarray(attention_mask, dtype=np.int32)
    U_np = np.ascontiguousarray(np.asarray(U, dtype=np.float32))
    w_np = np.ascontiguousarray(np.asarray(w, dtype=np.float32).reshape(-1))
    b_np = np.ascontiguousarray(np.asarray(b, dtype=np.float32).reshape(-1))

    in_maps = []
    for i in range(B):
        in_maps.append({
            "H": np.ascontiguousarray(H[i]),
            "mask": np.ascontiguousarray(attention_mask[i]),
            "U": U_np,
            "w": w_np,
            "b": b_np,
        })
    res = run_bass_kernel_spmd(nc, in_maps, list(range(B)))
    return np.stack([res.results[i]["out"] for i in range(B)], axis=0)


# revision 15
# speedup vs baseline: 1.5752x; 1.3386x over previous
"""Biaffine span head kernel for Trainium2 (Bass/Tile), SPMD over 8 NeuronCores.

Math (per batch element b):
    Hu   = H @ U                        [L, d]
    a    = H @ w1 + b                   [L]
    c    = H @ w2                       [L]
    bil[l, off] = <Hu[l, :], H[l+off, :]>        off in [0, 30)
    s    = bil + a[l] + c[l+off]
    m    = mask[l] * mask_pad[l+off]
    out[l, off] = s if m != 0 else -1e9          [L, 30]

Sharding: data-parallel over batch B=8 across the 8 cores (one batch row per
core); the H@U GEMM and the band loop are fully local per shard, no
collectives.

Per-core schedule: a software pipeline over 4 l-blocks of 512. For block lb:
H tiles are DMA'd in, transposed on the PE (4 transposes grouped into one
PSUM bank, one batched [128,512] copy out), then HuT = (H@U)^T accumulates
with fp32r (TF32) matmuls at 1 cycle/row, then the band Gram blocks
[128,158] run in fp32, and the 30-wide diagonal is extracted via a DRAM
bounce whose read access pattern has row stride 159 (diagonal = affine in
DRAM address space). The mask/bias fixup runs one block behind so every
dependency points backward in program order.
"""

import os
import sys

import numpy as np

for _p in ("/opt/trn_rl_repo",):
    if _p not in sys.path and os.path.isdir(_p):
        sys.path.insert(0, _p)

B = 8
L = 2048
D = 1024
K = 30          # band width (MAX_ANSWER_LEN)
P = 128         # partitions
NB = P + K      # 158: band matmul window
KC = D // P     # 8 contraction chunks
NBLK = 512      # l-block width
LBN = L // NBLK  # 4
TPB = NBLK // P  # 4 l-tiles per block
LPAD = L + 32   # padded H^T width (2080)
NEG = -1.0e9

# fp32r (TF32) runs the PE at 4x the fp32 rate for moving dims >= 256.
GEMM1_USE_F32R = True
# >1: wrap the compute phases in an on-device loop (perf measurement only)
PERF_ITERS = int(os.environ.get("BK_PERF_ITERS", "1"))
# perf-bisection switches (measurement only; leave unset for the real kernel)
SKIP_BAND = os.environ.get("BK_SKIP_BAND", "0") == "1"
SKIP_TR = os.environ.get("BK_SKIP_TR", "0") == "1"
SKIP_GEMM = os.environ.get("BK_SKIP_GEMM", "0") == "1"
SKIP_FIXUP = os.environ.get("BK_SKIP_FIXUP", "0") == "1" or SKIP_BAND

_CACHE = {}


def _build_nc():
    import contextlib

    import concourse.bass as bass
    import concourse.tile as tile
    from concourse import bacc, mybir

    f32 = mybir.dt.float32
    i32 = mybir.dt.int32
    gdt = mybir.dt.float32r if GEMM1_USE_F32R else f32

    nc = bacc.Bacc("TRN2", target_bir_lowering=False, debug=False, num_devices=B)

    H_h = nc.dram_tensor("H", [L, D], f32, kind="ExternalInput")
    mask_h = nc.dram_tensor("mask", [L], i32, kind="ExternalInput")
    U_h = nc.dram_tensor("U", [D, D], f32, kind="ExternalInput")
    w_h = nc.dram_tensor("w", [2 * D], f32, kind="ExternalInput")
    b_h = nc.dram_tensor("b", [1], f32, kind="ExternalInput")
    out_h = nc.dram_tensor("out", [L, K], f32, kind="ExternalOutput")

    band_h = nc.dram_tensor("band_scratch", [L // P, P, NB], f32)
    ac_h = nc.dram_tensor("ac_scratch", [2, LPAD], f32)      # row0 = a+b, row1 = c
    maskf_h = nc.dram_tensor("maskf_scratch", [LPAD], f32)

    ident_h = nc.inline_tensor(np.eye(P, dtype=np.float32), name="ident_const")

    H = H_h.ap()
    U = U_h.ap()
    out = out_h.ap()

    def dap(h, off, dims):
        # DRAM access pattern helper: dims = [(stride_elems, count), ...]
        return bass.AP(h, off, [list(d) for d in dims])

    with tile.TileContext(nc) as tc, tc.tile_pool(name="perm", bufs=1) as perm_pool:

        def perm(shape, dtype, name):
            return perm_pool.tile(shape, dtype, name=name, tag=name)

        def rnd(ap):
            # view through which a producer rounds values to fp32r precision
            return ap.bitcast(gdt) if GEMM1_USE_F32R else ap

        # ---- persistent SBUF tensors ----
        ident_sb = perm([P, P], f32, name="ident_sb")
        nc.sync.dma_start(ident_sb[:], ident_h.ap())

        U_sb = [perm([P, D], f32, name=f"U_sb{kk}") for kk in range(KC)]
        HT = [perm([P, LPAD], f32, name=f"HT{kk}") for kk in range(KC)]
        HuT = [perm([P, L], f32, name=f"HuT{kk}") for kk in range(KC)]
        w12 = perm([P, KC, 2], f32, name="w12")
        b_sb = perm([1, 1], f32, name="b_sb")
        zpad = perm([2, 32], f32, name="zpad")

        for kk in range(KC):
            nc.gpsimd.memset(HT[kk][:, L:LPAD], 0.0)
        nc.gpsimd.memset(zpad[:], 0.0)
        nc.sync.dma_start(dap(maskf_h, L, [(32, 1), (1, 32)]), zpad[0:1, :])
        nc.sync.dma_start(dap(ac_h, L, [(LPAD, 2), (1, 32)]), zpad[:, :])
        nc.sync.dma_start(b_sb[:], dap(b_h, 0, [(1, 1), (1, 1)]))

        loop_cm = (
            tc.For_i(0, PERF_ITERS, 1) if PERF_ITERS > 1 else contextlib.nullcontext()
        )
        with loop_cm, contextlib.ExitStack() as ctx:
            hstage_pool = ctx.enter_context(tc.tile_pool(name="hstage", bufs=5))
            wstage_pool = ctx.enter_context(tc.tile_pool(name="wstage", bufs=2))
            trps = ctx.enter_context(tc.tile_pool(name="trpsum", bufs=2, space="PSUM"))
            hups = ctx.enter_context(tc.tile_pool(name="hupsum", bufs=3, space="PSUM"))
            bps = ctx.enter_context(tc.tile_pool(name="bandpsum", bufs=2, space="PSUM"))
            acps = ctx.enter_context(tc.tile_pool(name="acpsum", bufs=1, space="PSUM"))
            acsb_pool = ctx.enter_context(tc.tile_pool(name="acsb", bufs=2))
            bsb_pool = ctx.enter_context(tc.tile_pool(name="bandsb", bufs=3))
            fix_pool = ctx.enter_context(tc.tile_pool(name="fix", bufs=2))
            mcast_pool = ctx.enter_context(tc.tile_pool(name="mcast", bufs=1))

            hstages = {}

            def load_block(lb):
                for i in range(TPB):
                    l0 = (lb * TPB + i) * P
                    hs = hstage_pool.tile([P, D], f32, name="hs", tag="hs")
                    nc.sync.dma_start(hs[:], H[l0:l0 + P, :])
                    hstages[(lb, i)] = hs

            def transposes(lb):
                j0 = lb * NBLK
                for kk in range(KC):
                    tp = trps.tile([P, NBLK], f32, name="tp", tag="tp")
                    for i in range(TPB):
                        nc.tensor.matmul(
                            tp[:, i * P:(i + 1) * P],
                            lhsT=hstages[(lb, i)][:, kk * P:(kk + 1) * P],
                            rhs=ident_sb[:],
                            is_transpose=True,
                            start=(i == 0),
                            stop=(i == TPB - 1),
                        )
                    nc.scalar.copy(rnd(HT[kk][:, j0:j0 + NBLK]), tp[:])
                for i in range(TPB):
                    del hstages[(lb, i)]

            def setup_weights():
                # emitted after block-0 H loads so the H DMAs go out first
                for kk in range(KC):
                    u_s = wstage_pool.tile([P, D], f32, name="u_s", tag="u_s")
                    nc.scalar.dma_start(u_s[:], U[kk * P:(kk + 1) * P, :])
                    if GEMM1_USE_F32R:
                        nc.scalar.copy(rnd(U_sb[kk][:]), u_s[:])
                    else:
                        nc.vector.tensor_copy(U_sb[kk][:], u_s[:])
                w_s = wstage_pool.tile([P, KC, 2], f32, name="w_s", tag="w_s")
                nc.sync.dma_start(w_s[:, :, 0], dap(w_h, 0, [(1, P), (P, KC)]))
                nc.sync.dma_start(w_s[:, :, 1], dap(w_h, D, [(1, P), (P, KC)]))
                nc.scalar.copy(rnd(w12[:]), w_s[:])
                m_i = mcast_pool.tile([P, L // P], i32, name="m_i")
                nc.sync.dma_start(m_i[:], dap(mask_h, 0, [(L // P, P), (1, L // P)]))
                m_f = mcast_pool.tile([P, L // P], f32, name="m_f")
                nc.vector.tensor_copy(m_f[:], m_i[:])
                nc.sync.dma_start(
                    dap(maskf_h, 0, [(L // P, P), (1, L // P)]), m_f[:]
                )

            def gemm1(lb):
                j0 = lb * NBLK
                for dc in range(KC):
                    hp = hups.tile([P, NBLK], f32, name="hp", tag="hp")
                    for kk in range(KC):
                        nc.tensor.matmul(
                            hp[:],
                            lhsT=rnd(U_sb[kk][:, dc * P:(dc + 1) * P]),
                            rhs=rnd(HT[kk][:, j0:j0 + NBLK]),
                            start=(kk == 0),
                            stop=(kk == KC - 1),
                        )
                    nc.vector.tensor_copy(HuT[dc][:, j0:j0 + NBLK], hp[:])

            def ac_gemm(lb):
                j0 = lb * NBLK
                acp = acps.tile([2, NBLK], f32, name="acp", tag="acp")
                for kk in range(KC):
                    nc.tensor.matmul(
                        acp[:],
                        lhsT=rnd(w12[:, kk, :]),
                        rhs=rnd(HT[kk][:, j0:j0 + NBLK]),
                        start=(kk == 0),
                        stop=(kk == KC - 1),
                    )
                ac_sb = acsb_pool.tile([2, NBLK], f32, name="ac_sb", tag="ac_sb")
                nc.vector.tensor_copy(ac_sb[:], acp[:])
                nc.vector.tensor_scalar_add(
                    ac_sb[0:1, :], ac_sb[0:1, :], b_sb[0:1, 0:1]
                )
                nc.scalar.dma_start(dap(ac_h, j0, [(LPAD, 2), (1, NBLK)]), ac_sb[:])

            def band_block(lb):
                bsb = bsb_pool.tile([P, TPB, NB], f32, name="bsb", tag="bsb")
                for i in range(TPB):
                    l0 = (lb * TPB + i) * P
                    bp = bps.tile([P, NB], f32, name="bp", tag="bp")
                    for kk in range(KC):
                        nc.tensor.matmul(
                            bp[:],
                            lhsT=HuT[kk][:, l0:l0 + P],
                            rhs=HT[kk][:, l0:l0 + NB],
                            start=(kk == 0),
                            stop=(kk == KC - 1),
                        )
                    nc.vector.tensor_copy(bsb[:, i, :], bp[:])
                nc.scalar.dma_start(
                    dap(band_h, lb * TPB * P * NB, [(NB, P), (P * NB, TPB), (1, NB)]),
                    bsb[:],
                )

            def fixup_block(lb):
                j0 = lb * NBLK
                bd_blk = fix_pool.tile([P, TPB, K], f32, name="bd_blk", tag="bd_blk")
                nc.sync.dma_start(
                    bd_blk[:],
                    dap(band_h, lb * TPB * P * NB, [(NB + 1, P), (P * NB, TPB), (1, K)]),
                )
                a_blk = fix_pool.tile([P, TPB], f32, name="a_blk", tag="a_blk")
                nc.sync.dma_start(a_blk[:], dap(ac_h, j0, [(1, P), (P, TPB)]))
                c_blk = fix_pool.tile([P, TPB, K], f32, name="c_blk", tag="c_blk")
                nc.sync.dma_start(
                    c_blk[:], dap(ac_h, LPAD + j0, [(1, P), (P, TPB), (1, K)])
                )
                md_blk = fix_pool.tile([P, TPB, K], f32, name="md_blk", tag="md_blk")
                nc.sync.dma_start(
                    md_blk[:], dap(maskf_h, j0, [(1, P), (P, TPB), (1, K)])
                )
                o_blk = fix_pool.tile([P, TPB, K], f32, name="o_blk", tag="o_blk")
                for i in range(TPB):
                    bd = bd_blk[:, i, :]
                    a_col = a_blk[:, i:i + 1]
                    c_d = c_blk[:, i, :]
                    md = md_blk[:, i, :]
                    mc = md_blk[:, i, 0:1]
                    f1 = fix_pool.tile([P, K], f32, name="f1", tag="f1")
                    # f1 = bil + a[l] + c[l+off]   (b already folded into a)
                    nc.vector.scalar_tensor_tensor(
                        f1[:], in0=bd, scalar=a_col, in1=c_d,
                        op0=mybir.AluOpType.add, op1=mybir.AluOpType.add,
                    )
                    # f2 = f1 * mask[l] * mask[l+off]
                    f2 = fix_pool.tile([P, K], f32, name="f2", tag="f2")
                    nc.vector.scalar_tensor_tensor(
                        f2[:], in0=f1[:], scalar=mc, in1=md,
                        op0=mybir.AluOpType.mult, op1=mybir.AluOpType.mult,
                    )
                    # f3 = m;  f4 = (m - 1) * 1e9  (0 kept / -1e9 masked)
                    f3 = fix_pool.tile([P, K], f32, name="f3", tag="f3")
                    nc.vector.tensor_scalar(
                        f3[:], in0=md, scalar1=mc, scalar2=None,
                        op0=mybir.AluOpType.mult,
                    )
                    f4 = fix_pool.tile([P, K], f32, name="f4", tag="f4")
                    nc.vector.tensor_scalar(
                        f4[:], in0=f3[:], scalar1=1.0, scalar2=-NEG,
                        op0=mybir.AluOpType.subtract, op1=mybir.AluOpType.mult,
                    )
                    nc.vector.tensor_add(o_blk[:, i, :], f2[:], f4[:])
                nc.sync.dma_start(
                    dap(out_h, lb * NBLK * K, [(K, P), (P * K, TPB), (1, K)]),
                    o_blk[:],
                )

            # ---- pipeline ----
            load_block(0)
            setup_weights()
            if not SKIP_TR:
                transposes(0)
            for lb in range(LBN):
                if lb + 1 < LBN:
                    load_block(lb + 1)
                    if not SKIP_TR:
                        transposes(lb + 1)
                if SKIP_TR:
                    for i in range(TPB):
                        hstages.pop((lb, i), None)
                if not SKIP_GEMM:
                    gemm1(lb)
                ac_gemm(lb)
                if not SKIP_BAND:
                    band_block(lb)
                if lb > 0 and not SKIP_FIXUP:
                    fixup_block(lb - 1)
            if not SKIP_FIXUP:
                fixup_block(LBN - 1)

    nc.compile()
    return nc


def get_nc():
    if "nc" not in _CACHE:
        _CACHE["nc"] = _build_nc()
    return _CACHE["nc"]


def kernel(H, attention_mask, U, w, b):
    from concourse.bass_utils import run_bass_kernel_spmd

    nc = get_nc()
    H = np.asarray(H, dtype=np.float32)
    attention_mask = np.asarray(attention_mask, dtype=np.int32)
    U_np = np.ascontiguousarray(np.asarray(U, dtype=np.float32))
    w_np = np.ascontiguousarray(np.asarray(w, dtype=np.float32).reshape(-1))
    b_np = np.ascontiguousarray(np.asarray(b, dtype=np.float32).reshape(-1))

    in_maps = []
    for i in range(B):
        in_maps.append({
            "H": np.ascontiguousarray(H[i]),
            "mask": np.ascontiguousarray(attention_mask[i]),
            "U": U_np,
            "w": w_np,
            "b": b_np,
        })
    res = run_bass_kernel_spmd(nc, in_maps, list(range(B)))
    return np.stack([res.results[i]["out"] for i in range(B)], axis=0)


# revision 17
# speedup vs baseline: 2.7162x; 1.7244x over previous
"""Biaffine span head kernel for Trainium2 (Bass/Tile), SPMD over 8 NeuronCores.

Math (per batch element b):
    Hu   = H @ U                        [L, d]
    a    = H @ w1 + b                   [L]
    c    = H @ w2                       [L]
    bil[l, off] = <Hu[l, :], H[l+off, :]>        off in [0, 30)
    s    = bil + a[l] + c[l+off]
    m    = mask[l] * mask_pad[l+off]
    out[l, off] = s if m != 0 else -1e9          [L, 30]

Sharding: data-parallel over batch B=8 across the 8 cores (one batch row per
core); the H@U GEMM and the band loop are fully local per shard, no
collectives.

Per-core schedule: a software pipeline over 4 l-blocks of 512. All matmul
operands are fp16 (same 10-bit mantissa as TF32, but 1 cycle/row on the PE
with fast weight loads); accumulation stays fp32 in PSUM. For block lb: H
tiles are DMA'd in, cast to fp16, transposed on the PE (4 transposes grouped
into one PSUM bank, one batched [128,512] copy out), then HuT = (H@U)^T
accumulates over 8 K-chunks, then the band Gram blocks [128,158] run. The
30-wide diagonal band is extracted via a DRAM bounce whose read access
pattern has row stride 159 (diagonal = affine in DRAM address space); the
mask/bias fixup runs once at the end on kernel-batched tiles using
broadcast (stride-0) operand views.
"""

import os
import sys

import numpy as np

for _p in ("/opt/trn_rl_repo",):
    if _p not in sys.path and os.path.isdir(_p):
        sys.path.insert(0, _p)

B = 8
L = 2048
D = 1024
K = 30          # band width (MAX_ANSWER_LEN)
P = 128         # partitions
NB = P + K      # 158: band matmul window
KC = D // P     # 8 contraction chunks
NBLK = 512      # l-block width
LBN = L // NBLK  # 4
TPB = NBLK // P  # 4 l-tiles per block
LT = L // P     # 16 l-tiles
LPAD = L + 32   # padded H^T width (2080)
NEG = -1.0e9

# >1: wrap the compute phases in an on-device loop (perf measurement only)
PERF_ITERS = int(os.environ.get("BK_PERF_ITERS", "1"))
# perf-bisection switches (measurement only; leave unset for the real kernel)
SKIP_BAND = os.environ.get("BK_SKIP_BAND", "0") == "1"
SKIP_TR = os.environ.get("BK_SKIP_TR", "0") == "1"
SKIP_GEMM = os.environ.get("BK_SKIP_GEMM", "0") == "1"
SKIP_FIXUP = os.environ.get("BK_SKIP_FIXUP", "0") == "1" or SKIP_BAND

_CACHE = {}


def _build_nc():
    import contextlib

    import concourse.bass as bass
    import concourse.tile as tile
    from concourse import bacc, mybir

    f32 = mybir.dt.float32
    f16 = mybir.dt.float16
    i32 = mybir.dt.int32

    nc = bacc.Bacc("TRN2", target_bir_lowering=False, debug=False, num_devices=B)

    H_h = nc.dram_tensor("H", [L, D], f32, kind="ExternalInput")
    mask_h = nc.dram_tensor("mask", [L], i32, kind="ExternalInput")
    U_h = nc.dram_tensor("U", [D, D], f32, kind="ExternalInput")
    w_h = nc.dram_tensor("w", [2 * D], f32, kind="ExternalInput")
    b_h = nc.dram_tensor("b", [1], f32, kind="ExternalInput")
    out_h = nc.dram_tensor("out", [L, K], f32, kind="ExternalOutput")

    band_h = nc.dram_tensor("band_scratch", [LT, P, NB], f32)
    ac_h = nc.dram_tensor("ac_scratch", [2, LPAD], f32)      # row0 = a+b, row1 = c
    maskf_h = nc.dram_tensor("maskf_scratch", [LPAD], f32)

    ident_h = nc.inline_tensor(np.eye(P, dtype=np.float16), name="ident_const")

    H = H_h.ap()

    def dap(h, off, dims):
        # DRAM access pattern helper: dims = [(stride_elems, count), ...]
        return bass.AP(h, off, [list(d) for d in dims])

    with tile.TileContext(nc) as tc, tc.tile_pool(name="perm", bufs=1) as perm_pool:

        def perm(shape, dtype, name):
            return perm_pool.tile(shape, dtype, name=name, tag=name)

        # ---- persistent SBUF tensors (fp16 datapath) ----
        ident_sb = perm([P, P], f16, name="ident_sb")
        nc.sync.dma_start(ident_sb[:], ident_h.ap())

        U16 = perm([P, KC, D], f16, name="U16")
        HT = [perm([P, LPAD], f16, name=f"HT{kk}") for kk in range(KC)]
        HuT = [perm([P, L], f16, name=f"HuT{kk}") for kk in range(KC)]
        w16 = perm([P, KC, 2], f16, name="w16")
        b_sb = perm([1, 1], f32, name="b_sb")
        zpad = perm([2, 32], f32, name="zpad")
        ac_all = perm([2, L], f32, name="ac_all")

        for kk in range(KC):
            nc.gpsimd.memset(HT[kk][:, L:LPAD], 0.0)
        nc.gpsimd.memset(zpad[:], 0.0)
        nc.sync.dma_start(dap(maskf_h, L, [(32, 1), (1, 32)]), zpad[0:1, :])
        nc.sync.dma_start(dap(ac_h, L, [(LPAD, 2), (1, 32)]), zpad[:, :])
        nc.sync.dma_start(b_sb[:], dap(b_h, 0, [(1, 1), (1, 1)]))

        loop_cm = (
            tc.For_i(0, PERF_ITERS, 1) if PERF_ITERS > 1 else contextlib.nullcontext()
        )
        with loop_cm, contextlib.ExitStack() as ctx:
            hstage_pool = ctx.enter_context(tc.tile_pool(name="hstage", bufs=5))
            h16_pool = ctx.enter_context(tc.tile_pool(name="h16", bufs=6))
            wstage_pool = ctx.enter_context(tc.tile_pool(name="wstage", bufs=1))
            trps = ctx.enter_context(tc.tile_pool(name="trpsum", bufs=2, space="PSUM"))
            hups = ctx.enter_context(tc.tile_pool(name="hupsum", bufs=3, space="PSUM"))
            bps = ctx.enter_context(tc.tile_pool(name="bandpsum", bufs=2, space="PSUM"))
            acps = ctx.enter_context(tc.tile_pool(name="acpsum", bufs=1, space="PSUM"))
            bsb_pool = ctx.enter_context(tc.tile_pool(name="bandsb", bufs=1))
            fix_pool = ctx.enter_context(tc.tile_pool(name="fix", bufs=1))
            mcast_pool = ctx.enter_context(tc.tile_pool(name="mcast", bufs=1))

            hstages = {}

            def load_block(lb):
                for i in range(TPB):
                    l0 = (lb * TPB + i) * P
                    hs = hstage_pool.tile([P, D], f32, name="hs", tag="hs")
                    nc.sync.dma_start(hs[:], H[l0:l0 + P, :])
                    h16 = h16_pool.tile([P, D], f16, name="h16t", tag="h16t")
                    nc.vector.tensor_copy(h16[:], hs[:])
                    hstages[(lb, i)] = h16

            def transposes(lb):
                j0 = lb * NBLK
                for kk in range(KC):
                    tp = trps.tile([P, NBLK], f16, name="tp", tag="tp")
                    for i in range(TPB):
                        nc.tensor.matmul(
                            tp[:, i * P:(i + 1) * P],
                            lhsT=hstages[(lb, i)][:, kk * P:(kk + 1) * P],
                            rhs=ident_sb[:],
                            is_transpose=True,
                            start=(i == 0),
                            stop=(i == TPB - 1),
                        )
                    nc.scalar.copy(HT[kk][:, j0:j0 + NBLK], tp[:])
                for i in range(TPB):
                    del hstages[(lb, i)]

            def setup_weights():
                # emitted after block-0 H loads so the H DMAs go out first
                for half in range(2):
                    u_s = wstage_pool.tile([P, 4, D], f32, name="u_s", tag="u_s")
                    nc.scalar.dma_start(
                        u_s[:],
                        dap(U_h, half * 4 * P * D, [(D, P), (P * D, 4), (1, D)]),
                    )
                    nc.scalar.copy(U16[:, half * 4:(half + 1) * 4, :], u_s[:])
                w_s = wstage_pool.tile([P, KC, 2], f32, name="w_s", tag="w_s")
                nc.scalar.dma_start(w_s[:, :, 0], dap(w_h, 0, [(1, P), (P, KC)]))
                nc.scalar.dma_start(w_s[:, :, 1], dap(w_h, D, [(1, P), (P, KC)]))
                nc.scalar.copy(w16[:], w_s[:])
                m_i = mcast_pool.tile([P, LT], i32, name="m_i")
                nc.scalar.dma_start(m_i[:], dap(mask_h, 0, [(LT, P), (1, LT)]))
                m_f = mcast_pool.tile([P, LT], f32, name="m_f")
                nc.vector.tensor_copy(m_f[:], m_i[:])
                nc.scalar.dma_start(dap(maskf_h, 0, [(LT, P), (1, LT)]), m_f[:])

            def gemm1(lb):
                j0 = lb * NBLK
                for dc in range(KC):
                    hp = hups.tile([P, NBLK], f32, name="hp", tag="hp")
                    for kk in range(KC):
                        nc.tensor.matmul(
                            hp[:],
                            lhsT=U16[:, kk, dc * P:(dc + 1) * P],
                            rhs=HT[kk][:, j0:j0 + NBLK],
                            start=(kk == 0),
                            stop=(kk == KC - 1),
                        )
                    nc.vector.tensor_copy(HuT[dc][:, j0:j0 + NBLK], hp[:])

            def ac_gemm(lb):
                j0 = lb * NBLK
                acp = acps.tile([2, NBLK], f32, name="acp", tag="acp")
                for kk in range(KC):
                    nc.tensor.matmul(
                        acp[:],
                        lhsT=w16[:, kk, :],
                        rhs=HT[kk][:, j0:j0 + NBLK],
                        start=(kk == 0),
                        stop=(kk == KC - 1),
                    )
                nc.vector.tensor_copy(ac_all[:, j0:j0 + NBLK], acp[:])
                nc.vector.tensor_scalar_add(
                    ac_all[0:1, j0:j0 + NBLK], ac_all[0:1, j0:j0 + NBLK],
                    b_sb[0:1, 0:1],
                )

            def band(lb, bsb, slot):
                for i in range(TPB):
                    l0 = (lb * TPB + i) * P
                    bp = bps.tile([P, NB], f32, name="bp", tag="bp")
                    for kk in range(KC):
                        nc.tensor.matmul(
                            bp[:],
                            lhsT=HuT[kk][:, l0:l0 + P],
                            rhs=HT[kk][:, l0:l0 + NB],
                            start=(kk == 0),
                            stop=(kk == KC - 1),
                        )
                    nc.vector.tensor_copy(bsb[:, slot * TPB + i, :], bp[:])

            def fixup_all():
                # batched loads for all 16 l-tiles; diagonal reads are affine
                bd = fix_pool.tile([P, LT, K], f32, name="bd", tag="bd")
                nc.sync.dma_start(
                    bd[:], dap(band_h, 0, [(NB + 1, P), (P * NB, LT), (1, K)])
                )
                a_t = fix_pool.tile([P, LT], f32, name="a_t", tag="a_t")
                nc.scalar.dma_start(a_t[:], dap(ac_h, 0, [(1, P), (P, LT)]))
                c_d = fix_pool.tile([P, LT, K], f32, name="c_d", tag="c_d")
                nc.scalar.dma_start(
                    c_d[:], dap(ac_h, LPAD, [(1, P), (P, LT), (1, K)])
                )
                md = fix_pool.tile([P, LT, K], f32, name="md", tag="md")
                nc.sync.dma_start(
                    md[:], dap(maskf_h, 0, [(1, P), (P, LT), (1, K)])
                )
                # broadcast views: repeat along the K axis with stride 0
                a_bc = bass.AP(a_t.tensor, a_t.offset,
                               list(a_t.ap[:1]) + [[1, LT], [0, K]])
                mc_bc = bass.AP(md.tensor, md.offset,
                                list(md.ap[:1]) + [[K, LT], [0, K]])
                f1 = fix_pool.tile([P, LT, K], f32, name="f1", tag="f1")
                nc.vector.tensor_add(f1[:], bd[:], c_d[:])          # bil + c
                f2 = fix_pool.tile([P, LT, K], f32, name="f2", tag="f2")
                nc.vector.tensor_add(f2[:], f1[:], a_bc)            # + a (+b)
                m_all = fix_pool.tile([P, LT, K], f32, name="m_all", tag="m_all")
                nc.vector.tensor_mul(m_all[:], md[:], mc_bc)        # m = m_l * m_loff
                f3 = fix_pool.tile([P, LT, K], f32, name="f3", tag="f3")
                nc.vector.tensor_mul(f3[:], f2[:], m_all[:])        # s * m
                f4 = fix_pool.tile([P, LT, K], f32, name="f4", tag="f4")
                nc.vector.tensor_scalar(                            # (m-1)*1e9
                    f4[:], in0=m_all[:], scalar1=1.0, scalar2=-NEG,
                    op0=mybir.AluOpType.subtract, op1=mybir.AluOpType.mult,
                )
                o_all = fix_pool.tile([P, LT, K], f32, name="o_all", tag="o_all")
                nc.vector.tensor_add(o_all[:], f3[:], f4[:])
                nc.sync.dma_start(
                    dap(out_h, 0, [(K, P), (P * K, LT), (1, K)]), o_all[:]
                )

            # ---- pipeline ----
            load_block(0)
            setup_weights()
            if not SKIP_TR:
                transposes(0)
            bsbs = [
                bsb_pool.tile([P, 2 * TPB, NB], f32, name=f"bsb{h}", tag=f"bsb{h}")
                for h in range(2)
            ]
            for lb in range(LBN):
                if lb + 1 < LBN:
                    load_block(lb + 1)
                    if not SKIP_TR:
                        transposes(lb + 1)
                if SKIP_TR:
                    for i in range(TPB):
                        hstages.pop((lb, i), None)
                if not SKIP_GEMM:
                    gemm1(lb)
                ac_gemm(lb)
                if not SKIP_BAND:
                    band(lb, bsbs[lb // 2], lb % 2)
                    if lb % 2 == 1:  # dump 8 tiles per DMA
                        nc.scalar.dma_start(
                            dap(band_h, (lb - 1) * TPB * P * NB,
                                [(NB, P), (P * NB, 2 * TPB), (1, NB)]),
                            bsbs[lb // 2][:],
                        )
            nc.scalar.dma_start(dap(ac_h, 0, [(LPAD, 2), (1, L)]), ac_all[:])
            if not SKIP_FIXUP:
                fixup_all()

    nc.compile()
    return nc


def get_nc():
    if "nc" not in _CACHE:
        _CACHE["nc"] = _build_nc()
    return _CACHE["nc"]


def kernel(H, attention_mask, U, w, b):
    from concourse.bass_utils import run_bass_kernel_spmd

    nc = get_nc()
    H = np.asarray(H, dtype=np.float32)
    attention_mask = np.asarray(attention_mask, dtype=np.int32)
    U_np = np.ascontiguousarray(np.asarray(U, dtype=np.float32))
    w_np = np.ascontiguousarray(np.asarray(w, dtype=np.float32).reshape(-1))
    b_np = np.ascontiguousarray(np.asarray(b, dtype=np.float32).reshape(-1))

    in_maps = []
    for i in range(B):
        in_maps.append({
            "H": np.ascontiguousarray(H[i]),
            "mask": np.ascontiguousarray(attention_mask[i]),
            "U": U_np,
            "w": w_np,
            "b": b_np,
        })
    res = run_bass_kernel_spmd(nc, in_maps, list(range(B)))
    return np.stack([res.results[i]["out"] for i in range(B)], axis=0)
